# revision 1
# baseline (speedup 1.0000x reference)
"""FlowNetC correlation kernel for Trainium2 (8 NeuronCores, data-parallel over batch).

Problem: out[b, d, y, x] = (1/256) * sum_c in1[b,c,y,x] * in2pad[b,c,y+dy,x+dx]
  with in2 zero-padded by 20 on each spatial side, d = 21*dyi + dxi,
  dy = 2*dyi - 20, dx = 2*dxi - 20 (21x21 = 441 displacements, stride 2).
Shapes: in1/in2 [16, 256, 48, 64] f32 -> out [16, 441, 48, 64] f32.

Strategy per core (2 images):
  - Parity-split rows/cols (displacements are stride-2, so x couples only to
    same-parity padded cols).  24 blocks per image of M=128 = 8 same-parity
    rows x 16 same-parity cols.  PE computes Gram blocks
    G[m=(ys,x_e), n=(rs,u)] = sum_c A[c,y,x] * B[c,rp,xp] as bf16 matmuls
    (1 cycle/row vs 4 for fp32; inputs are cast to bf16 on-chip, which also
    halves staging SBUF and scratch DMA traffic).  Weights are packed into a
    per-block-contiguous apk tile (PE weight APs allow only one free dim).
  - DVE/Act copy PSUM into a per-image bf16 staging tile sg with the 24
    blocks INTERLEAVED innermost ([(rs,u)][blk]), folding the 1/256
    normalization into the copy; never-written pad-u stripes are pre-zeroed
    once.  Image 1's prep (casts+packs) runs on Pool so it never blocks
    image 0's scatter stream; image 1's scatters run DVE-only so image 0's
    shuffles (Act+Pool) aren't starved.
  - Row-diagonal applied at WRITE time: per (image, ys) one DMA writes only
    the 21-row rs window [ys, ys+21) of partitions m=(ys,:) to DRAM scratch
    (75% of sg, vs 100% for a full dump).  Scratch layout [ys][x_e][dyi][u*blk].
  - Column-diagonal applied at GATHER time: u = x_e + dxi is an affine offset
    on the DRAM side; with blk innermost (dxi, blk) merge into 1008B runs.
    One gather DMA per (image, ys) into s3 tiles with partitions = (slice,
    dyi) - four (b, ys) slices packed per 84-partition group.
  - Act/Pool (img 0) / DVE/Act/Pool (img 1) free-dim shuffle to x-contiguous
    order into per-yt s4 tiles, then final DMAs to the d-major output with
    512B runs.  Loads for image 1 are emitted right after image 0's prep so
    their transfers fill DMA idle time during image 0's matmul phase.

Cost-model timeline per core: 128.6us (from 219.0us baseline).  DMA busy
~105.9us is the binding resource (loads 35 + scratch write 25.8 + gather 15
+ output 30); PE ~39us, engines ~30-45us each, all hidden under DMA.
"""

import numpy as np

_CACHE = {}

# ---- geometry (hardcoded for [16, 256, 48, 64]) ----
N_CORES = 8
B2 = 2            # images per core
CH = 2            # channel chunks
CP = 128          # channels per chunk (partition dim)
H, W = 48, 64
PY, YT, YS = 2, 3, 8     # y parity, y tiles, rows per block
PX, XT, XE = 2, 2, 16    # x parity, x tiles, cols per block
NDI = 21                 # dyi / dxi count
RS, U = 28, 36           # rhs window rows / cols (parity space)
NBLK = PY * YT * PX * XT          # 24 blocks per image
RP = 88                           # padded in2 rows
NN = RS * U                       # 1008 sg free rows (rs,u)
SGF = NN * NBLK                   # sg free size per partition = 24192
WRUN = NDI * U * NBLK             # write run per partition = 18144
SCRS = XE * WRUN                  # scratch elems per (b, ys) = 169344
S3B = XE * NDI * NBLK             # s3 free = 8064
S4F = NDI * PY * W                # per-yt s4 free = 2688


def _build():
    import concourse.bacc as bacc
    import concourse.bass as bass
    import concourse.mybir as mybir
    import concourse.tile as tile

    f32 = mybir.dt.float32
    f32r = mybir.dt.float32r
    bf16 = mybir.dt.bfloat16
    COPY = mybir.ActivationFunctionType.Copy
    nc = bacc.Bacc("TRN2", target_bir_lowering=False, debug=False,
                   enable_asserts=False, num_devices=N_CORES)

    in1 = nc.dram_tensor("in1", [B2, CH * CP, H, W], f32, kind="ExternalInput")
    in2 = nc.dram_tensor("in2", [B2, CH * CP, H, W], f32, kind="ExternalInput")
    out = nc.dram_tensor("out", [B2, NDI * NDI, H, W], f32, kind="ExternalOutput")

    with tile.TileContext(nc) as tc:
        with (
            tc.tile_pool(name="scr", bufs=1, space="DRAM") as scr_pool,
            tc.tile_pool(name="io", bufs=1) as io_pool,
            tc.tile_pool(name="s3p", bufs=1) as s3_pool,
            tc.tile_pool(name="apkp", bufs=1) as apk_pool,
            tc.tile_pool(name="s4p", bufs=1) as s4_pool,
            tc.tile_pool(name="psum", bufs=4, space="PSUM") as psum_pool,
        ):
            scrs = []
            for b in range(B2):
                scr_b = scr_pool.tile([128, YS * SCRS // 128], bf16,
                                      tag=f"scr{b}")
                scrs.append(scr_b)
            stg1 = io_pool.tile([CP, CH * H * W], f32)     # 24.6KB/part
            stg2 = io_pool.tile([CP, CH * H * W], f32)     # 24.6KB/part
            bsb = io_pool.tile([CP, CH, RP, W], bf16)      # 22.5KB/part
            sg = io_pool.tile([CP, SGF], bf16)             # 47.3KB/part

            # one-time zeroing: in2 pad rows; sg pad-u stripes (never written
            # by scatters: xt=0 blocks own u in [0,10), xt=1 u in [26,36))
            nc.gpsimd.memset(bsb[:, :, 0:20, :], 0.0)
            nc.gpsimd.memset(bsb[:, :, 68:88, :], 0.0)
            nc.gpsimd.memset(
                bass.AP(sg.tensor, 0,
                        [[SGF, CP], [2, NBLK // 2], [U * NBLK, RS], [NBLK, 10]]),
                0.0)
            nc.gpsimd.memset(
                bass.AP(sg.tensor, 26 * NBLK + 1,
                        [[SGF, CP], [2, NBLK // 2], [U * NBLK, RS], [NBLK, 10]]),
                0.0)

            dmae = [nc.sync, nc.scalar]

            def emit_loads(b):
                nc.sync.dma_start(
                    bass.AP(stg1.tensor, 0,
                            [[CH * H * W, CP], [H * W, CH], [1, H * W]]),
                    bass.AP(in1, b * CH * CP * H * W,
                            [[H * W, CP], [CP * H * W, CH], [1, H * W]]))
                nc.sync.dma_start(
                    bass.AP(stg2.tensor, 0,
                            [[CH * H * W, CP], [H * W, CH], [1, H * W]]),
                    bass.AP(in2, b * CH * CP * H * W,
                            [[H * W, CP], [CP * H * W, CH], [1, H * W]]))

            def emit_prep(b, apk):
                """casts + weight packs for image b.  b=0 runs on DVE+Act
                (start of kernel, both idle); b=1 runs on Pool only so it
                never blocks image 0's scatter/shuffle stream."""
                engs = [nc.vector, nc.scalar] if b == 0 else [nc.gpsimd]
                k = 0
                for ch in range(CH):
                    for py in range(PY):
                        for px in range(PX):
                            psrc = bass.AP(
                                stg1.tensor, ch * H * W + py * W + px,
                                [[CH * H * W, CP], [32, XT],
                                 [2 * W, YT * YS], [2, XE]])
                            pdst = bass.AP(
                                apk.tensor,
                                ((ch * PY + py) * PX + px) * XT * YT * 128,
                                [[CH * PY * PX * XT * YT * 128, CP],
                                 [YT * 128, XT], [16, YT * YS], [1, XE]])
                            e = engs[k % len(engs)]; k += 1
                            if e is nc.scalar:
                                nc.scalar.activation(pdst, psrc, COPY)
                            else:
                                e.tensor_copy(pdst, psrc)
                for ch in range(CH):
                    c2dst = bsb[:, ch, 20:20 + H, :]
                    c2src = bass.AP(stg2.tensor, ch * H * W,
                                    [[CH * H * W, CP], [1, H * W]])
                    e = engs[k % len(engs)]; k += 1
                    if e is nc.scalar:
                        nc.scalar.activation(c2dst, c2src, COPY)
                    else:
                        e.tensor_copy(c2dst, c2src)

            def emit_blocks(b, apk):
                """matmuls + PSUM->sg scatters for image b (PE; DVE+Act)."""
                for py in range(PY):
                    for yt in range(YT):
                        y0 = yt * 16 + py
                        for px in range(PX):
                            for xt in range(XT):
                                x0 = xt * 32 + px
                                blk = ((py * YT + yt) * PX + px) * XT + xt
                                u_lo = 10 if xt == 0 else 0
                                xp0 = x0 + 2 * u_lo - 20
                                ps = psum_pool.tile([128, 1024], f32)
                                for ch in range(CH):
                                    lt = apk[:, ch, py, px, xt, yt, :]
                                    for h in range(2):
                                        rhs = bsb[:, ch,
                                                  y0 + 28 * h:y0 + 28 * h + 27:2,
                                                  xp0:xp0 + 51:2]
                                        nc.tensor.matmul(
                                            ps[:, 512 * h:512 * h + 364],
                                            lt, rhs,
                                            start=(ch == 0), stop=(ch == CH - 1))
                                csrc = bass.AP(ps.tensor, 0,
                                               [[1024, 128], [512, 2],
                                                [26, 14], [1, 26]])
                                cdst = bass.AP(sg.tensor, u_lo * NBLK + blk,
                                               [[SGF, 128], [14 * U * NBLK, 2],
                                                [U * NBLK, 14], [NBLK, 26]])
                                if b == 1 or blk % 2 == 0:
                                    nc.vector.tensor_scalar_mul(
                                        cdst, csrc, 1.0 / 256.0)
                                else:
                                    nc.scalar.activation(
                                        cdst, csrc, COPY, scale=1.0 / 256.0)

            def emit_writes(b):
                """sg rs-windows -> DRAM scratch; one DMA per ys, on SP."""
                for ys in range(YS):
                    wsrc = sg[16 * ys:16 * ys + 16,
                              ys * U * NBLK:ys * U * NBLK + WRUN]
                    wdst = bass.AP(scrs[b].tensor, ys * SCRS,
                                   [[WRUN, XE], [1, WRUN]])
                    nc.sync.dma_start(wdst, wsrc)

            def emit_gathers(b):
                """scratch -> s3 band gathers (diag in u), on Act; returns
                the two 4-slice-group s3 tiles."""
                s3s = []
                for g in range(2):
                    s3 = s3_pool.tile([4 * NDI, S3B], bf16, tag=f"s3_{g}")
                    for s in range(4):
                        ys = g * 4 + s
                        gsrc = bass.AP(scrs[b].tensor, ys * SCRS,
                                       [[U * NBLK, NDI],
                                        [WRUN + NBLK, XE],
                                        [1, NDI * NBLK]])
                        gdst = bass.AP(s3.tensor, s * NDI * S3B,
                                       [[S3B, NDI], [NDI * NBLK, XE],
                                        [1, NDI * NBLK]])
                        nc.scalar.dma_start(gdst, gsrc)
                    s3s.append(s3)
                return s3s

            def emit_shuffles(b, s3s):
                """s3 -> s4 x-interleave + bf16->f32 cast, then final DMAs."""
                cpe = 0
                for g in range(2):
                    s3 = s3s[g]
                    for yt in range(YT):
                        s4 = s4_pool.tile([4 * NDI, S4F], f32,
                                          tag=f"s4_{(g * YT + yt) % 3}")
                        for py in range(PY):
                            for px in range(PX):
                                blk0 = ((py * YT + yt) * PX + px) * XT
                                ssrc = bass.AP(
                                    s3.tensor, blk0,
                                    [[S3B, 4 * NDI],
                                     [NDI * NBLK, XE],
                                     [1, XT],
                                     [NBLK, NDI]])
                                sdst = bass.AP(
                                    s4.tensor, py * W + px,
                                    [[S4F, 4 * NDI],
                                     [2, XE],
                                     [32, XT],
                                     [PY * W, NDI]])
                                if b == 0:
                                    eng = 1 + cpe % 2   # Act/Pool only
                                else:
                                    eng = cpe % 3
                                cpe += 1
                                if eng == 0:
                                    nc.vector.tensor_copy(sdst, ssrc)
                                elif eng == 1:
                                    nc.scalar.activation(sdst, ssrc, COPY)
                                else:
                                    nc.gpsimd.tensor_copy(sdst, ssrc)
                        for s in range(4):
                            ys = g * 4 + s
                            y = yt * 16 + 2 * ys
                            fsrc = bass.AP(s4.tensor, s * NDI * S4F,
                                           [[S4F, NDI], [PY * W, NDI],
                                            [1, PY * W]])
                            fdst = bass.AP(out, b * 441 * H * W + y * W,
                                           [[NDI * H * W, NDI],
                                            [H * W, NDI],
                                            [1, PY * W]])
                            dmae[(s + yt) % 2].dma_start(fdst, fsrc)

            apks = []
            for b in range(B2):
                apk_b = apk_pool.tile([CP, CH, PY, PX, XT, YT, 128], bf16,
                                      tag=f"apk{b}")
                apks.append(apk_b)

            emit_loads(0)
            emit_prep(0, apks[0])       # DVE+Act; packs free stg1 first
            emit_loads(1)               # WAR on stg1/stg2: fires after prep_b0
            emit_blocks(0, apks[0])
            emit_prep(1, apks[1])       # Pool only; runs during image 0
            emit_writes(0)
            g0 = emit_gathers(0)
            emit_blocks(1, apks[1])     # DVE/Act scatters before b0 shuffles
            emit_shuffles(0, g0)
            emit_writes(1)
            g1 = emit_gathers(1)
            emit_shuffles(1, g1)

    nc.compile()
    return nc


def _get_nc():
    if "nc" not in _CACHE:
        _CACHE["nc"] = _build()
    return _CACHE["nc"]


def kernel(input1, input2):
    from concourse.bass_utils import run_bass_kernel_spmd

    input1 = np.ascontiguousarray(np.asarray(input1), dtype=np.float32)
    input2 = np.ascontiguousarray(np.asarray(input2), dtype=np.float32)
    nc = _get_nc()
    in_maps = [
        {"in1": input1[i * B2:(i + 1) * B2], "in2": input2[i * B2:(i + 1) * B2]}
        for i in range(N_CORES)
    ]
    res = run_bass_kernel_spmd(nc, in_maps, list(range(N_CORES)))
    return np.concatenate([res.results[i]["out"] for i in range(N_CORES)], axis=0)

